# revision 5
# baseline (speedup 1.0000x reference)
"""Spatio-temporal Hawkes process log-likelihood on Trainium2 (Bass/Tile).

Computes, for x[B, L, 3] = (t, s1, s2) and scalars mu/alpha/beta/sigma:
  lams[b, i]  = softplus(sum_{j<i} K(x_i, x_j) * 1[t_j>0] + mu) + 1e-5
  loglik[b]   = sum_i log(lams[b,i]) * 1[t_i>0]
              - UNIT_VOL * sum_{r, g} softplus(sum_j K((tt_r, ss_g), x_j) * m + mu)
with K(x, y) = norm * exp(-beta*(t_x - t_y) - |s_x - s_y|^2 / (2 sigma^2)),
norm = alpha*beta/(2 pi sigma^2), over a 50 x 50 x 50 (t, s1, s2) grid.

Strategy (one batch element per NeuronCore, 8 cores, data-parallel):
  The grid kernel factorizes: exp(-beta*(tt_r - t_j)) * exp(-ds2/2sig^2).
  Per core build G[j, g] = exp(-inv2sig2 * |ss_g - s_j|^2)  (via a K=5
  matmul computing the quadratic expansion of ds2, the per-event s^2
  term riding in the ACT bias), and
  W[j, r] = norm * 1[0 < t_j <= tt_r] * exp(beta*(t_j - tt_r)).
  Then softplus-arg = W.T @ G on the PE, softplus+row-sum fused on
  ACT/DVE, and ones-matmuls reduce partitions.
  The per-event [L, L] exponent is built by 4 accumulated rank-1
  matmuls (outer sums + cross terms), one ACT exp, and a masked
  row-reduce fused into a scalar_tensor_tensor.

  Partition packing: the 2500 spatial grid points are split in two
  halves of 1250; partitions 0:64 hold events-vs-half0, 64:128 hold
  events-vs-half1, so elementwise engines run at full 128-lane width.

softplus is decomposed as relu(x) + log1p(exp(-|x|)) so every ACT func
(Exp, Ln, Copy) lives in the single `natural_log_exp_and_others` table
set -> one activation-table load.
"""

import math
import numpy as np
from contextlib import ExitStack

R = 50                      # INT_RES (time and each spatial axis)
RG = R * R                  # 2500 spatial grid points
HALF = RG // 2              # 1250
NCORES = 8
UNIT_VOL = 1.0 / float(R ** 3)
BIG_NEG = 1.0e30

_prog_cache: dict = {}


def _const_arrays(L: int, norm: float):
    f32 = np.float32
    g1 = np.linspace(0.0, 1.0, R).astype(f32)
    g2 = np.linspace(0.0, 1.0, R).astype(f32)
    sg = np.arange(RG)
    c1 = g1[sg // R]            # [2500]
    c2 = g2[sg % R]             # [2500]
    lo, hi = slice(0, HALF), slice(HALF, RG)
    grid_rhs = np.stack([
        c1[lo] ** 2 + c2[lo] ** 2,
        c1[hi] ** 2 + c2[hi] ** 2,
        -2.0 * c1[lo],
        -2.0 * c1[hi],
        -2.0 * c2[lo],
    ]).astype(f32)              # [5, 1250]

    tril_n = (norm * np.tril(np.ones((L, L), np.float64), -1)).astype(f32)

    ccols = np.zeros((128, 2), f32)
    ccols[:, 0] = 1.0                                   # ones col
    ccols[:, 1] = (np.arange(128) % 64 < R)             # sel col (valid r rows)

    ones_row = np.ones((1, 128), f32)
    neg_ttg = (-np.linspace(0.0, 1.0, R)).astype(f32)[None, :]   # [1, 50]
    return dict(grid_rhs=grid_rhs, tril_n=tril_n, ccols=ccols,
                ones_row=ones_row, neg_ttg=neg_ttg)


def _marshal_core_inputs(t, s1, s2):
    """Pure-layout staging of one sequence's inputs (no arithmetic)."""
    f32 = np.float32
    L = t.shape[0]
    cols = np.zeros((128, 3), f32)
    cols[0:L, 0] = t; cols[64:64 + L, 0] = t
    cols[0:L, 1] = s1; cols[64:64 + L, 1] = s1
    cols[0:L, 2] = s2; cols[64:64 + L, 2] = s2
    rows = np.zeros((1, 3 * L), f32)
    rows[0, 0:L] = t; rows[0, L:2 * L] = s1; rows[0, 2 * L:3 * L] = s2
    lhs5 = np.zeros((5, 128), f32)
    lhs5[0, 0:64] = 1.0
    lhs5[1, 64:128] = 1.0
    lhs5[2, 0:L] = s1
    lhs5[3, 64:64 + L] = s1
    lhs5[4, 0:L] = s2; lhs5[4, 64:64 + L] = s2
    return {"cols_in": cols, "rows_in": rows, "lhs5_in": lhs5}


def _build_program(mu: float, beta: float, inv2sig2: float, norm: float, L: int):
    import concourse.bass as bass
    import concourse.bacc as bacc
    import concourse.tile as tile
    import concourse.mybir as mybir

    f32 = mybir.dt.float32
    Act = mybir.ActivationFunctionType
    Op = mybir.AluOpType

    nc = bacc.Bacc("TRN2", target_bir_lowering=False, debug=False,
                   enable_asserts=True, num_devices=NCORES)

    # ---- DRAM I/O
    cols_d = nc.dram_tensor("cols_in", [128, 3], f32, kind="ExternalInput").ap()
    rows_d = nc.dram_tensor("rows_in", [1, 3 * L], f32, kind="ExternalInput").ap()
    lhs5_d = nc.dram_tensor("lhs5_in", [5, 128], f32, kind="ExternalInput").ap()
    grid_rhs_d = nc.dram_tensor("grid_rhs", [5, HALF], f32, kind="ExternalInput").ap()
    tril_d = nc.dram_tensor("tril_n", [L, L], f32, kind="ExternalInput").ap()
    ccols_d = nc.dram_tensor("ccols", [128, 2], f32, kind="ExternalInput").ap()
    ones_d = nc.dram_tensor("ones_row", [1, 128], f32, kind="ExternalInput").ap()
    negttg_d = nc.dram_tensor("neg_ttg", [1, R], f32, kind="ExternalInput").ap()
    lams_o = nc.dram_tensor("lams_o", [L], f32, kind="ExternalOutput").ap()
    ll_o = nc.dram_tensor("ll_o", [1], f32, kind="ExternalOutput").ap()

    with tile.TileContext(nc) as tc, ExitStack() as ctx:
        pool = ctx.enter_context(tc.tile_pool(name="sbuf", bufs=1))
        psum = ctx.enter_context(tc.tile_pool(name="psum", bufs=1,
                                              space=bass.MemorySpace.PSUM))

        # ---- loads
        rhs5 = pool.tile([5, HALF], f32)
        nc.sync.dma_start(rhs5[:], grid_rhs_d[:])
        tril = pool.tile([L, L], f32)
        nc.sync.dma_start(tril[:], tril_d[:])
        ccols = pool.tile([128, 2], f32)
        nc.sync.dma_start(ccols[:], ccols_d[:])
        ones_r = pool.tile([1, 128], f32)
        nc.sync.dma_start(ones_r[:], ones_d[:])
        negttg = pool.tile([1, R], f32)
        nc.sync.dma_start(negttg[:], negttg_d[:])
        ctile = pool.tile([128, 3], f32)
        nc.sync.dma_start(ctile[:], cols_d[:])
        rtile = pool.tile([1, 3 * L], f32)
        nc.sync.dma_start(rtile[:], rows_d[:])
        lhsT5 = pool.tile([5, 128], f32)
        nc.sync.dma_start(lhsT5[:], lhs5_d[:])

        t_col = ctile[:, 0:1]
        s1_col = ctile[:, 1:2]
        s2_col = ctile[:, 2:3]
        t_row = rtile[0:1, 0:L]
        s1_row = rtile[0:1, L:2 * L]
        s2_row = rtile[0:1, 2 * L:3 * L]

        mu_col = pool.tile([128, 1], f32)
        nc.vector.memset(mu_col[:], mu)

        # ---- per-partition spatial bias: -inv2sig2 * (s1^2 + s2^2)
        c1sq = pool.tile([128, 1], f32)
        nc.vector.tensor_tensor(c1sq[:], s1_col, s1_col, Op.mult)
        c2sq = pool.tile([128, 1], f32)
        nc.vector.tensor_tensor(c2sq[:], s2_col, s2_col, Op.mult)
        ssq = pool.tile([128, 1], f32)
        nc.vector.tensor_tensor(ssq[:], c1sq[:], c2sq[:], Op.add)
        gbias = pool.tile([128, 1], f32)
        nc.vector.tensor_scalar(gbias[:], ssq[:], -inv2sig2, None, Op.mult)

        # ---- grid spatial kernel G[j(packed), g] = exp(-inv2sig2 * ds2)
        ds2_ps = psum.tile([128, 1536], f32, tag="big")
        for off, w in ((0, 512), (512, 512), (1024, HALF - 1024)):
            nc.tensor.matmul(ds2_ps[:, off:off + w], lhsT5[0:5, :],
                             rhs5[0:5, off:off + w], start=True, stop=True)
        G = pool.tile([128, HALF], f32)
        nc.scalar.activation(G[:], ds2_ps[:, 0:HALF], Act.Exp,
                             scale=-inv2sig2, bias=gbias[:, 0:1])

        # ---- temporal weights W_T[j(packed), r] (norm folded into mask)
        bc_ps = psum.tile([128, R], f32, tag="bc")
        nc.tensor.matmul(bc_ps[:], ones_r[:], negttg[:], start=True, stop=True)
        dtW = pool.tile([128, R], f32)
        nc.vector.tensor_scalar(dtW[:], bc_ps[:], t_col, None, Op.add)
        Ew = pool.tile([128, R], f32)
        nc.scalar.activation(Ew[:], dtW[:], Act.Exp, scale=beta)
        hn_col = pool.tile([128, 1], f32)
        nc.vector.tensor_scalar(hn_col[:], t_col, 0.0, norm, Op.is_gt, Op.mult)
        h_col = pool.tile([128, 1], f32)
        nc.vector.tensor_scalar(h_col[:], t_col, 0.0, None, Op.is_gt)
        Mw = pool.tile([128, R], f32)
        nc.vector.tensor_scalar(Mw[:], dtW[:], 0.0, hn_col[:, 0:1], Op.is_le, Op.mult)
        WT2 = pool.tile([128, 64], f32)
        nc.vector.memset(WT2[:], 0.0)
        nc.vector.tensor_tensor(WT2[:, 0:R], Ew[:], Mw[:], Op.mult)

        # ---- softplus-arg = W.T @ G   [r-packed, g]
        z_ps = psum.tile([128, 1536], f32, tag="big")
        for h in (0, 1):
            p0 = h * 64
            for off, w in ((0, 512), (512, 512), (1024, HALF - 1024)):
                nc.tensor.matmul(z_ps[p0:p0 + 64, off:off + w],
                                 WT2[p0:p0 + 64, 0:64],
                                 G[p0:p0 + 64, off:off + w],
                                 start=True, stop=True)

        # ---- softplus(z + mu) summed over grid; relu + log1p(exp(-|.|))
        r_sb = pool.tile([128, HALF], f32)
        nc.vector.tensor_scalar(r_sb[:], z_ps[:, 0:HALF], mu, 0.0, Op.add, Op.max)
        a_sb = pool.tile([128, HALF], f32)
        nc.scalar.activation(a_sb[:], z_ps[:, 0:HALF], Act.Abs, bias=mu_col[:, 0:1])
        e_sb = pool.tile([128, HALF], f32)
        nc.scalar.activation(e_sb[:], a_sb[:], Act.Exp, scale=-1.0)
        l_sb = pool.tile([128, HALF], f32)
        nc.scalar.activation(l_sb[:], e_sb[:], Act.Ln, bias=1.0)
        sp_sb = pool.tile([128, HALF], f32)
        int_col = pool.tile([128, 1], f32)
        nc.vector.scalar_tensor_tensor(sp_sb[:], l_sb[:], 0.0, r_sb[:],
                                       Op.add, Op.add, accum_out=int_col[:])

        red_ps = psum.tile([1, 2], f32, tag="red")
        nc.tensor.matmul(red_ps[0:1, 1:2], int_col[:, 0:1], ccols[:, 1:2],
                         start=True, stop=True)

        # ---- per-event exponent via 4 accumulated rank-1 matmuls
        sq1 = pool.tile([1, L], f32)
        nc.vector.tensor_tensor(sq1[:], s1_row, s1_row, Op.mult)
        sq2 = pool.tile([1, L], f32)
        nc.vector.tensor_tensor(sq2[:], s2_row, s2_row, Op.mult)
        ssum = pool.tile([1, L], f32)
        nc.vector.tensor_tensor(ssum[:], sq1[:], sq2[:], Op.add)
        w_row = pool.tile([1, L], f32)
        nc.vector.tensor_scalar(w_row[:], ssum[:], -inv2sig2, None, Op.mult)

        u_row = pool.tile([1, L], f32)
        nc.vector.scalar_tensor_tensor(u_row[:], t_row, -beta, w_row[:],
                                       Op.mult, Op.add)
        v_row = pool.tile([1, L], f32)
        nc.vector.scalar_tensor_tensor(v_row[:], t_row, beta, w_row[:],
                                       Op.mult, Op.add)
        ha = pool.tile([1, L], f32)
        nc.vector.tensor_scalar(ha[:], t_row, 0.0, 1.0, Op.is_gt, Op.subtract)
        hm = pool.tile([1, L], f32)
        nc.vector.tensor_scalar(hm[:], ha[:], BIG_NEG, None, Op.mult)
        vp_row = pool.tile([1, L], f32)
        nc.vector.tensor_tensor(vp_row[:], v_row[:], hm[:], Op.add)
        a1_row = pool.tile([1, L], f32)
        nc.vector.tensor_scalar(a1_row[:], s1_row, 2.0 * inv2sig2, None, Op.mult)
        a2_row = pool.tile([1, L], f32)
        nc.vector.tensor_scalar(a2_row[:], s2_row, 2.0 * inv2sig2, None, Op.mult)

        zev_ps = psum.tile([L, L], f32, tag="zev")
        nc.tensor.matmul(zev_ps[:], u_row[:], ones_r[0:1, 0:L],
                         start=True, stop=False)
        nc.tensor.matmul(zev_ps[:], ones_r[0:1, 0:L], vp_row[:],
                         start=False, stop=False)
        nc.tensor.matmul(zev_ps[:], a1_row[:], s1_row,
                         start=False, stop=False)
        nc.tensor.matmul(zev_ps[:], a2_row[:], s2_row,
                         start=False, stop=True)

        Ke = pool.tile([L, L], f32)
        nc.scalar.activation(Ke[:], zev_ps[:], Act.Exp)
        Km = pool.tile([L, L], f32)
        lam_col = pool.tile([L, 1], f32)
        nc.vector.scalar_tensor_tensor(Km[:], Ke[:], 0.0, tril[:],
                                       Op.add, Op.mult, accum_out=lam_col[:])

        # lams = softplus(lam_raw + mu) + 1e-5 ; log-lik pieces
        rr = pool.tile([L, 1], f32)
        nc.vector.tensor_scalar(rr[:], lam_col[:], mu, 0.0, Op.add, Op.max)
        aa = pool.tile([L, 1], f32)
        nc.scalar.activation(aa[:], lam_col[:], Act.Abs, bias=mu_col[0:L, 0:1])
        ee = pool.tile([L, 1], f32)
        nc.scalar.activation(ee[:], aa[:], Act.Exp, scale=-1.0)
        lp = pool.tile([L, 1], f32)
        nc.scalar.activation(lp[:], ee[:], Act.Ln, bias=1.0)
        lams_col = pool.tile([L, 1], f32)
        nc.vector.scalar_tensor_tensor(lams_col[:], lp[:], 1e-5, rr[:],
                                       Op.add, Op.add)
        nc.sync.dma_start(lams_o.unsqueeze(1), lams_col[:])

        lnl = pool.tile([L, 1], f32)
        nc.scalar.activation(lnl[:], lams_col[:], Act.Ln)
        lnm = pool.tile([L, 1], f32)
        nc.vector.tensor_tensor(lnm[:], lnl[:], h_col[0:L, 0:1], Op.mult)
        nc.tensor.matmul(red_ps[0:1, 0:1], lnm[:, 0:1], ccols[0:L, 0:1],
                         start=True, stop=True)

        sl_sb = pool.tile([1, 1], f32)
        nc.scalar.copy(sl_sb[:], red_ps[0:1, 0:1])
        ll_sb = pool.tile([1, 1], f32)
        nc.vector.scalar_tensor_tensor(ll_sb[:], red_ps[0:1, 1:2], -UNIT_VOL,
                                       sl_sb[:], Op.mult, Op.add)
        nc.sync.dma_start(ll_o.unsqueeze(1), ll_sb[:])

    nc.compile()
    return nc


def _get_program(mu, alpha, beta, sigma, L):
    key = (float(mu), float(alpha), float(beta), float(sigma), L)
    if key not in _prog_cache:
        sig2 = float(sigma) * float(sigma)
        inv2sig2 = 1.0 / (2.0 * sig2)
        norm = float(alpha) * float(beta) / (2.0 * math.pi * sig2)
        nc = _build_program(float(mu), float(beta), inv2sig2, norm, L)
        consts = _const_arrays(L, norm)
        _prog_cache[key] = (nc, consts)
    return _prog_cache[key]


def kernel(x, mu, alpha, beta, sigma):
    from concourse.bass_utils import run_bass_kernel_spmd

    x = np.asarray(x, dtype=np.float32)
    B, L, _ = x.shape
    assert B == NCORES, f"expected batch {NCORES}, got {B}"

    nc, consts = _get_program(mu, alpha, beta, sigma, L)

    in_maps = []
    for b in range(B):
        m = dict(consts)
        m.update(_marshal_core_inputs(x[b, :, 0], x[b, :, 1], x[b, :, 2]))
        in_maps.append(m)

    res = run_bass_kernel_spmd(nc, in_maps, list(range(NCORES)))
    lams = np.stack([res.results[b]["lams_o"] for b in range(B)]).astype(np.float32)
    loglik = np.stack([res.results[b]["ll_o"][0] for b in range(B)]).astype(np.float32)
    return lams, loglik


# revision 7
# speedup vs baseline: 1.1658x; 1.1658x over previous
"""Spatio-temporal Hawkes process log-likelihood on Trainium2 (Bass/Tile).

Computes, for x[B, L, 3] = (t, s1, s2) and scalars mu/alpha/beta/sigma:
  lams[b, i]  = softplus(sum_{j<i} K(x_i, x_j) * 1[t_j>0] + mu) + 1e-5
  loglik[b]   = sum_i log(lams[b,i]) * 1[t_i>0]
              - UNIT_VOL * sum_{r, g} softplus(sum_j K((tt_r, ss_g), x_j) * m + mu)
with K(x, y) = norm * exp(-beta*(t_x - t_y) - |s_x - s_y|^2 / (2 sigma^2)),
norm = alpha*beta/(2 pi sigma^2), over a 50 x 50 x 50 (t, s1, s2) grid.

Strategy (one batch element per NeuronCore, 8 cores, data-parallel):
  The grid kernel factorizes: exp(-beta*(tt_r - t_j)) * exp(-ds2/2sig^2).
  Per core build G[j, g] = exp(-inv2sig2 * |ss_g - s_j|^2)  (via a K=5
  matmul computing the quadratic expansion of ds2, the per-event s^2
  term riding in the ACT bias), and
  W[j, r] = norm * 1[0 < t_j <= tt_r] * exp(beta*(t_j - tt_r)).
  Then softplus-arg = W.T @ G on the PE (bf16 operands - the outputs
  only feed softplus+sum, fp32 accumulation in PSUM), with the
  softplus+row-sum stage chunk-pipelined against the matmuls.
  The per-event [L, L] exponent is built by 4 accumulated rank-1 fp32
  matmuls (outer sums + cross terms; fp32 because the expansion
  cancels catastrophically in low precision), one ACT exp, and a
  masked row-reduce fused into a scalar_tensor_tensor.

  Partition packing: the 2500 spatial grid points are split in two
  halves of 1250; partitions 0:64 hold events-vs-half0, 64:128 hold
  events-vs-half1, so elementwise engines run at full 128-lane width.

softplus is decomposed as relu(v) + log1p(exp(-|v|)) with
|v| = 2*relu(v) - v (DVE) so the only ACT funcs are Exp/Ln/Copy; the
activation-table map is patched during compile so every func resolves
to the single `natural_log_exp_and_others` set -> one table load.

All tiny per-core staging (dup columns, concatenated rows, the K=5
lhsT) is marshalled host-side as pure copies - engines can only
address SBUF partition starts of 0/32/64/96, so single-row writes at
other partitions are not expressible on-device.
"""

import math
import numpy as np
from contextlib import ExitStack

R = 50                      # INT_RES (time and each spatial axis)
RG = R * R                  # 2500 spatial grid points
HALF = RG // 2              # 1250
NCORES = 8
UNIT_VOL = 1.0 / float(R ** 3)
BIG_NEG = 1.0e30
CHUNKS = ((0, 512), (512, 512), (1024, HALF - 1024))

_prog_cache: dict = {}


def _const_arrays(L: int, norm: float):
    f32 = np.float32
    g1 = np.linspace(0.0, 1.0, R).astype(f32)
    g2 = np.linspace(0.0, 1.0, R).astype(f32)
    sg = np.arange(RG)
    c1 = g1[sg // R]            # [2500]
    c2 = g2[sg % R]             # [2500]
    lo, hi = slice(0, HALF), slice(HALF, RG)
    grid_rhs = np.stack([
        c1[lo] ** 2 + c2[lo] ** 2,
        c1[hi] ** 2 + c2[hi] ** 2,
        -2.0 * c1[lo],
        -2.0 * c1[hi],
        -2.0 * c2[lo],
    ]).astype(f32)              # [5, 1250]

    # const blob [128, 256]:
    #   [0:64, 0:64]   tril_n (norm * strict lower triangular)
    #   [:, 64:65]     ones column
    #   [:, 65:66]     sel column (valid packed r rows)
    #   [0:1, 66:194]  ones row (128)
    #   [0:1, 194:244] -linspace(0,1,R)
    cblob = np.zeros((128, 256), f32)
    cblob[0:L, 0:L] = norm * np.tril(np.ones((L, L), np.float64), -1)
    cblob[:, 64] = 1.0
    cblob[:, 65] = (np.arange(128) % 64 < R)
    cblob[0, 66:194] = 1.0
    cblob[0, 194:244] = -np.linspace(0.0, 1.0, R)
    return dict(grid_rhs=grid_rhs, cblob=cblob)


def _marshal_core_inputs(t, s1, s2):
    """Pure-layout staging of one sequence's inputs (no arithmetic).

    iblob [128, 336]:
      [:, 0:3]       t/s1/s2 duplicated into both partition halves
      [0:1, 3:195]   t | s1 | s2 concatenated rows
      [0:5, 195:323] K=5 lhsT for the ds2 matmul:
                     [ind_lo; ind_hi; s1*ind_lo; s1*ind_hi; s2_dup]
    """
    f32 = np.float32
    L = t.shape[0]
    blob = np.zeros((128, 336), f32)
    blob[0:L, 0] = t; blob[64:64 + L, 0] = t
    blob[0:L, 1] = s1; blob[64:64 + L, 1] = s1
    blob[0:L, 2] = s2; blob[64:64 + L, 2] = s2
    blob[0, 3:3 + L] = t
    blob[0, 3 + L:3 + 2 * L] = s1
    blob[0, 3 + 2 * L:3 + 3 * L] = s2
    blob[0, 195:195 + 64] = 1.0                        # ind_lo
    blob[1, 195 + 64:195 + 128] = 1.0                  # ind_hi
    blob[2, 195:195 + L] = s1
    blob[3, 195 + 64:195 + 64 + L] = s1
    blob[4, 195:195 + L] = s2
    blob[4, 195 + 64:195 + 64 + L] = s2
    return {"iblob": blob}


def _patched_act_tables(orig_fn, preferred="natural_log_exp_and_others"):
    """Wrap get_activation_tables so every function present in the
    preferred set resolves only to it (same names/order, so the emitted
    act_func_set_id still indexes the real act_info.json)."""
    import functools

    @functools.cache
    def wrapper(arch):
        tables = dict(orig_fn(arch))
        pref = tables.get(preferred)
        if not pref:
            return tables
        return {
            name: (funcs if name == preferred else funcs - pref)
            for name, funcs in tables.items()
        }
    return wrapper


def _build_program(mu: float, beta: float, inv2sig2: float, norm: float, L: int):
    import concourse.bass as bass
    import concourse.bacc as bacc
    import concourse.tile as tile
    import concourse.mybir as mybir

    f32 = mybir.dt.float32
    bf16 = mybir.dt.bfloat16
    Act = mybir.ActivationFunctionType
    Op = mybir.AluOpType

    nc = bacc.Bacc("TRN2", target_bir_lowering=False, debug=False,
                   enable_asserts=True, num_devices=NCORES)

    # ---- DRAM I/O
    iblob_d = nc.dram_tensor("iblob", [128, 336], f32, kind="ExternalInput").ap()
    grid_rhs_d = nc.dram_tensor("grid_rhs", [5, HALF], f32, kind="ExternalInput").ap()
    cblob_d = nc.dram_tensor("cblob", [128, 256], f32, kind="ExternalInput").ap()
    lams_o = nc.dram_tensor("lams_o", [L], f32, kind="ExternalOutput").ap()
    ll_o = nc.dram_tensor("ll_o", [1], f32, kind="ExternalOutput").ap()

    with tile.TileContext(nc) as tc, ExitStack() as ctx:
        pool = ctx.enter_context(tc.tile_pool(name="sbuf", bufs=1))
        cpool = ctx.enter_context(tc.tile_pool(name="chunk", bufs=2))
        psum = ctx.enter_context(tc.tile_pool(name="psum", bufs=1,
                                              space=bass.MemorySpace.PSUM))
        psmall = ctx.enter_context(tc.tile_pool(name="psmall", bufs=2,
                                                space=bass.MemorySpace.PSUM))

        # ---- loads (3 DMAs on 2 queues)
        iblob = pool.tile([128, 336], f32)
        nc.sync.dma_start(iblob[:], iblob_d[:])
        cblob = pool.tile([128, 256], f32)
        nc.gpsimd.dma_start(cblob[:], cblob_d[:])
        rhs5 = pool.tile([5, HALF], f32)
        nc.gpsimd.dma_start(rhs5[:], grid_rhs_d[:])

        t_col = iblob[:, 0:1]
        s1_col = iblob[:, 1:2]
        s2_col = iblob[:, 2:3]
        t_row = iblob[0:1, 3:3 + L]
        s1_row = iblob[0:1, 3 + L:3 + 2 * L]
        s2_row = iblob[0:1, 3 + 2 * L:3 + 3 * L]
        lhsT5 = iblob[0:5, 195:323]

        tril = cblob[0:L, 0:L]
        ones_col = cblob[:, 64:65]
        sel_col = cblob[:, 65:66]
        ones_r = cblob[0:1, 66:194]
        ones_r64 = cblob[0:1, 66:130]
        negttg = cblob[0:1, 194:244]

        mu_col = pool.tile([128, 1], f32)
        nc.vector.memset(mu_col[:], mu)

        # ---- per-partition spatial bias: -inv2sig2 * (s1^2 + s2^2)
        c2sq = pool.tile([128, 1], f32)
        nc.vector.tensor_scalar(c2sq[:], s2_col, s2_col, None, Op.mult)
        ssq = pool.tile([128, 1], f32)
        nc.vector.scalar_tensor_tensor(ssq[:], s1_col, s1_col, c2sq[:],
                                       Op.mult, Op.add)
        gbias = pool.tile([128, 1], f32)
        nc.vector.tensor_scalar(gbias[:], ssq[:], -inv2sig2, None, Op.mult)

        # ---- temporal weights W_T[j(packed), r] (norm folded into mask)
        bc_ps = psmall.tile([128, R], f32, tag="small")
        nc.tensor.matmul(bc_ps[:], ones_r[:], negttg[:], start=True, stop=True)
        dtW = pool.tile([128, R], f32)
        nc.vector.tensor_scalar(dtW[:], bc_ps[:], t_col, None, Op.add)
        Ew = pool.tile([128, R], f32)
        nc.scalar.activation(Ew[:], dtW[:], Act.Exp, scale=beta)
        hn_col = pool.tile([128, 1], f32)
        nc.vector.tensor_scalar(hn_col[:], t_col, 0.0, norm, Op.is_gt, Op.mult)
        h_col = pool.tile([128, 1], f32)
        nc.vector.tensor_scalar(h_col[:], t_col, 0.0, None, Op.is_gt)
        Mw = pool.tile([128, R], f32)
        nc.vector.tensor_scalar(Mw[:], dtW[:], 0.0, hn_col[:, 0:1], Op.is_le, Op.mult)
        WT2 = pool.tile([128, 64], bf16)
        nc.vector.memset(WT2[:], 0.0)
        nc.vector.tensor_tensor(WT2[:, 0:R], Ew[:], Mw[:], Op.mult)

        # ---- grid: per 512-chunk pipeline of
        #      ds2 matmul -> G=exp -> W.T@G -> softplus+accumulate
        acc = pool.tile([128, 4], f32)
        G = pool.tile([128, HALF], bf16)
        ds2_ps = psum.tile([128, 1536], f32, tag="ds2")
        z_ps = psum.tile([128, 1536], f32, tag="z")
        for ci, (off, w) in enumerate(CHUNKS):
            nc.tensor.matmul(ds2_ps[:, off:off + w], lhsT5,
                             rhs5[0:5, off:off + w], start=True, stop=True)
            nc.scalar.activation(G[:, off:off + w], ds2_ps[:, off:off + w],
                                 Act.Exp, scale=-inv2sig2, bias=gbias[:, 0:1])
            for h in (0, 1):
                p0 = h * 64
                nc.tensor.matmul(z_ps[p0:p0 + 64, off:off + w],
                                 WT2[p0:p0 + 64, 0:64],
                                 G[p0:p0 + 64, off:off + w],
                                 start=True, stop=True)
            # softplus(v)+sum, v = z+mu: r=relu(v); exp(-(2r-z-mu)); log1p; +r
            r_c = cpool.tile([128, 512], f32, tag="r_c")
            nc.vector.tensor_scalar(r_c[:, 0:w], z_ps[:, off:off + w],
                                    mu, 0.0, Op.add, Op.max)
            t_c = cpool.tile([128, 512], f32, tag="t_c")
            nc.vector.scalar_tensor_tensor(t_c[:, 0:w], r_c[:, 0:w], 2.0,
                                           z_ps[:, off:off + w],
                                           Op.mult, Op.subtract)
            e_c = cpool.tile([128, 512], f32, tag="e_c")
            nc.scalar.activation(e_c[:, 0:w], t_c[:, 0:w], Act.Exp,
                                 scale=-1.0, bias=mu_col[:, 0:1])
            l_c = cpool.tile([128, 512], f32, tag="l_c")
            nc.scalar.activation(l_c[:, 0:w], e_c[:, 0:w], Act.Ln, bias=1.0)
            sp_c = cpool.tile([128, 512], f32, tag="sp_c")
            nc.vector.scalar_tensor_tensor(sp_c[:, 0:w], l_c[:, 0:w], 0.0,
                                           r_c[:, 0:w], Op.add, Op.add,
                                           accum_out=acc[:, ci:ci + 1])

        int_col = pool.tile([128, 1], f32)
        nc.vector.tensor_reduce(int_col[:], acc[:, 0:3],
                                mybir.AxisListType.X, Op.add)
        red_ps = psmall.tile([1, 2], f32, tag="small")
        nc.tensor.matmul(red_ps[0:1, 1:2], int_col[:, 0:1], sel_col,
                         start=True, stop=True)

        # ---- per-event exponent via 4 accumulated rank-1 matmuls (fp32)
        sq1 = pool.tile([1, L], f32)
        nc.vector.tensor_tensor(sq1[:], s1_row, s1_row, Op.mult)
        sq2 = pool.tile([1, L], f32)
        nc.vector.tensor_tensor(sq2[:], s2_row, s2_row, Op.mult)
        ssum = pool.tile([1, L], f32)
        nc.vector.tensor_tensor(ssum[:], sq1[:], sq2[:], Op.add)
        w_row = pool.tile([1, L], f32)
        nc.vector.tensor_scalar(w_row[:], ssum[:], -inv2sig2, None, Op.mult)

        u_row = pool.tile([1, L], f32)
        nc.vector.scalar_tensor_tensor(u_row[:], t_row, -beta, w_row[:],
                                       Op.mult, Op.add)
        v_row = pool.tile([1, L], f32)
        nc.vector.scalar_tensor_tensor(v_row[:], t_row, beta, w_row[:],
                                       Op.mult, Op.add)
        ha = pool.tile([1, L], f32)
        nc.vector.tensor_scalar(ha[:], t_row, 0.0, 1.0, Op.is_gt, Op.subtract)
        hm = pool.tile([1, L], f32)
        nc.vector.tensor_scalar(hm[:], ha[:], BIG_NEG, None, Op.mult)
        vp_row = pool.tile([1, L], f32)
        nc.vector.tensor_tensor(vp_row[:], v_row[:], hm[:], Op.add)
        a1_row = pool.tile([1, L], f32)
        nc.vector.tensor_scalar(a1_row[:], s1_row, 2.0 * inv2sig2, None, Op.mult)
        a2_row = pool.tile([1, L], f32)
        nc.vector.tensor_scalar(a2_row[:], s2_row, 2.0 * inv2sig2, None, Op.mult)

        zev_ps = psmall.tile([L, L], f32, tag="small")
        nc.tensor.matmul(zev_ps[:], u_row[:], ones_r64, start=True, stop=False)
        nc.tensor.matmul(zev_ps[:], ones_r64, vp_row[:], start=False, stop=False)
        nc.tensor.matmul(zev_ps[:], a1_row[:], s1_row, start=False, stop=False)
        nc.tensor.matmul(zev_ps[:], a2_row[:], s2_row, start=False, stop=True)

        Ke = pool.tile([L, L], f32)
        nc.scalar.activation(Ke[:], zev_ps[:], Act.Exp)
        Km = pool.tile([L, L], f32)
        lam_col = pool.tile([L, 1], f32)
        nc.vector.scalar_tensor_tensor(Km[:], Ke[:], 0.0, tril,
                                       Op.add, Op.mult, accum_out=lam_col[:])

        # lams = softplus(lam_raw + mu) + 1e-5 ; log-lik pieces
        rr = pool.tile([L, 1], f32)
        nc.vector.tensor_scalar(rr[:], lam_col[:], mu, 0.0, Op.add, Op.max)
        tt_ = pool.tile([L, 1], f32)
        nc.vector.scalar_tensor_tensor(tt_[:], rr[:], 2.0, lam_col[:],
                                       Op.mult, Op.subtract)
        ee = pool.tile([L, 1], f32)
        nc.scalar.activation(ee[:], tt_[:], Act.Exp, scale=-1.0,
                             bias=mu_col[0:L, 0:1])
        lp = pool.tile([L, 1], f32)
        nc.scalar.activation(lp[:], ee[:], Act.Ln, bias=1.0)
        lams_col = pool.tile([L, 1], f32)
        nc.vector.scalar_tensor_tensor(lams_col[:], lp[:], 1e-5, rr[:],
                                       Op.add, Op.add)
        nc.sync.dma_start(lams_o.unsqueeze(1), lams_col[:])

        lnl = pool.tile([L, 1], f32)
        nc.scalar.activation(lnl[:], lams_col[:], Act.Ln)
        lnm = pool.tile([L, 1], f32)
        nc.vector.tensor_tensor(lnm[:], lnl[:], h_col[0:L, 0:1], Op.mult)
        nc.tensor.matmul(red_ps[0:1, 0:1], lnm[:, 0:1], ones_col[0:L, 0:1],
                         start=True, stop=True)

        sl_sb = pool.tile([1, 1], f32)
        nc.scalar.copy(sl_sb[:], red_ps[0:1, 0:1])
        ll_sb = pool.tile([1, 1], f32)
        nc.vector.scalar_tensor_tensor(ll_sb[:], red_ps[0:1, 1:2], -UNIT_VOL,
                                       sl_sb[:], Op.mult, Op.add)
        nc.sync.dma_start(ll_o.unsqueeze(1), ll_sb[:])

    import concourse.hw_specs as hw_specs
    orig = bacc.get_activation_tables
    bacc.get_activation_tables = _patched_act_tables(hw_specs.get_activation_tables)
    try:
        nc.compile()
    finally:
        bacc.get_activation_tables = orig
    return nc


def _get_program(mu, alpha, beta, sigma, L):
    key = (float(mu), float(alpha), float(beta), float(sigma), L)
    if key not in _prog_cache:
        sig2 = float(sigma) * float(sigma)
        inv2sig2 = 1.0 / (2.0 * sig2)
        norm = float(alpha) * float(beta) / (2.0 * math.pi * sig2)
        nc = _build_program(float(mu), float(beta), inv2sig2, norm, L)
        consts = _const_arrays(L, norm)
        _prog_cache[key] = (nc, consts)
    return _prog_cache[key]


def kernel(x, mu, alpha, beta, sigma):
    from concourse.bass_utils import run_bass_kernel_spmd

    x = np.asarray(x, dtype=np.float32)
    B, L, _ = x.shape
    assert B == NCORES, f"expected batch {NCORES}, got {B}"

    nc, consts = _get_program(mu, alpha, beta, sigma, L)

    in_maps = []
    for b in range(B):
        m = dict(consts)
        m.update(_marshal_core_inputs(x[b, :, 0], x[b, :, 1], x[b, :, 2]))
        in_maps.append(m)

    res = run_bass_kernel_spmd(nc, in_maps, list(range(NCORES)))
    lams = np.stack([res.results[b]["lams_o"] for b in range(B)]).astype(np.float32)
    loglik = np.stack([res.results[b]["ll_o"][0] for b in range(B)]).astype(np.float32)
    return lams, loglik


# revision 12
# speedup vs baseline: 1.4346x; 1.2306x over previous
"""Spatio-temporal Hawkes process log-likelihood on Trainium2 (Bass/Tile).

Computes, for x[B, L, 3] = (t, s1, s2) and scalars mu/alpha/beta/sigma:
  lams[b, i]  = softplus(sum_{j<i} K(x_i, x_j) * 1[t_j>0] + mu) + 1e-5
  loglik[b]   = sum_i log(lams[b,i]) * 1[t_i>0]
              - UNIT_VOL * sum_{r, g} softplus(sum_j K((tt_r, ss_g), x_j) * m + mu)
with K(x, y) = norm * exp(-beta*(t_x - t_y) - |s_x - s_y|^2 / (2 sigma^2)),
norm = alpha*beta/(2 pi sigma^2), over a 50 x 50 x 50 (t, s1, s2) grid.

Strategy (one batch element per NeuronCore, 8 cores, data-parallel):
  The grid kernel factorizes: exp(-beta*(tt_r - t_j)) * exp(-ds2/2sig^2).
  Per core build G[j, g] = exp(-inv2sig2 * |ss_g - s_j|^2)  (via a K=5
  matmul computing the quadratic expansion of ds2, the per-event s^2
  term riding in the ACT bias), and
  W[j, r] = norm * 1[0 < t_j <= tt_r] * exp(beta*(t_j - tt_r)).
  Then softplus-arg = W.T @ G on the PE (bf16 operands - the outputs
  only feed softplus+sum, fp32 accumulation in PSUM), with the
  softplus+row-sum stage chunk-pipelined against the matmuls.
  The per-event [L, L] exponent is built by 4 accumulated rank-1 fp32
  matmuls (outer sums + cross terms; fp32 because the expansion
  cancels catastrophically in low precision), one ACT exp, and a
  masked row-reduce fused into a scalar_tensor_tensor.

  Partition packing: the 2500 spatial grid points are split in two
  halves of 1250; partitions 0:64 hold events-vs-half0, 64:128 hold
  events-vs-half1, so elementwise engines run at full 128-lane width.

softplus is decomposed as relu(v) + log1p(exp(-|v|)) with
|v| = 2*relu(v) - v (DVE) so the only ACT funcs are Exp/Ln/Copy; the
activation-table map is patched during compile so every func resolves
to the single `natural_log_exp_and_others` set -> one table load.

All tiny per-core staging (dup columns, concatenated rows, the K=5
lhsT) is marshalled host-side as pure copies - engines can only
address SBUF partition starts of 0/32/64/96, so single-row writes at
other partitions are not expressible on-device.
"""

import math
import numpy as np
from contextlib import ExitStack

R = 50                      # INT_RES (time and each spatial axis)
RG = R * R                  # 2500 spatial grid points
HALF = RG // 2              # 1250
NCORES = 8
UNIT_VOL = 1.0 / float(R ** 3)
BIG_NEG = 1.0e30
CHUNKS = ((0, 512), (512, 512), (1024, HALF - 1024))

_prog_cache: dict = {}


def _const_arrays(L: int, norm: float):
    f32 = np.float32
    g1 = np.linspace(0.0, 1.0, R).astype(f32)
    g2 = np.linspace(0.0, 1.0, R).astype(f32)
    sg = np.arange(RG)
    c1 = g1[sg // R]            # [2500]
    c2 = g2[sg % R]             # [2500]
    lo, hi = slice(0, HALF), slice(HALF, RG)
    grid_rhs = np.stack([
        c1[lo] ** 2 + c2[lo] ** 2,
        c1[hi] ** 2 + c2[hi] ** 2,
        -2.0 * c1[lo],
        -2.0 * c1[hi],
        -2.0 * c2[lo],
    ]).astype(f32)              # [5, 1250]

    # const blob [128, 256]:
    #   [0:64, 0:64]   tril_n (norm * strict lower triangular)
    #   [:, 64:65]     ones column
    #   [:, 65:66]     sel column (valid packed r rows)
    #   [0:1, 66:194]  ones row (128)
    #   [0:1, 194:244] -linspace(0,1,R)
    cblob = np.zeros((128, 256), f32)
    cblob[0:L, 0:L] = norm * np.tril(np.ones((L, L), np.float64), -1)
    cblob[:, 64] = 1.0
    cblob[:, 65] = (np.arange(128) % 64 < R)
    cblob[0, 66:194] = 1.0
    cblob[0, 194:244] = -np.linspace(0.0, 1.0, R)
    return dict(grid_rhs=grid_rhs, cblob=cblob)


def _scalar_consts(beta: float, inv2sig2: float):
    f32 = np.float32
    sc = np.zeros((2, 2), f32)
    sc[0, 0] = -beta; sc[1, 0] = 2.0 * inv2sig2        # colA -> [u; a1]
    sc[0, 1] = beta;  sc[1, 1] = 2.0 * inv2sig2        # colB -> [v; a2]
    return sc


def _marshal_core_inputs(t, s1, s2):
    """Pure-layout staging of one sequence's inputs (no arithmetic).

    iblob [128, 600]:
      [:, 0:3]       t/s1/s2 duplicated into both partition halves
      [0:1, 3:195]   t | s1 | s2 concatenated rows
      [0:5, 195:323] K=5 lhsT for the ds2 matmul:
                     [ind_lo; ind_hi; s1*ind_lo; s1*ind_hi; s2_dup]
      [0:2, 336:400] [t; s1]   (pair-packed per-event row inputs)
      [0:2, 400:464] [t; s2]
      [0:2, 464:528] [ones; s1] (rhs of per-event matmul 1)
      [0:2, 528:592] [ones; s2] (lhsT of per-event matmul 2)
    """
    f32 = np.float32
    L = t.shape[0]
    blob = np.zeros((128, 600), f32)
    blob[0:L, 0] = t; blob[64:64 + L, 0] = t
    blob[0:L, 1] = s1; blob[64:64 + L, 1] = s1
    blob[0:L, 2] = s2; blob[64:64 + L, 2] = s2
    blob[0, 3:3 + L] = t
    blob[0, 3 + L:3 + 2 * L] = s1
    blob[0, 3 + 2 * L:3 + 3 * L] = s2
    blob[0, 195:195 + 64] = 1.0                        # ind_lo
    blob[1, 195 + 64:195 + 128] = 1.0                  # ind_hi
    blob[2, 195:195 + L] = s1
    blob[3, 195 + 64:195 + 64 + L] = s1
    blob[4, 195:195 + L] = s2
    blob[4, 195 + 64:195 + 64 + L] = s2
    blob[0, 336:336 + L] = t;    blob[1, 336:336 + L] = s1
    blob[0, 400:400 + L] = t;    blob[1, 400:400 + L] = s2
    blob[0, 464:464 + L] = 1.0;  blob[1, 464:464 + L] = s1
    blob[0, 528:528 + L] = 1.0;  blob[1, 528:528 + L] = s2
    return {"iblob": blob}


def _patched_act_tables(orig_fn, preferred="natural_log_exp_and_others"):
    """Wrap get_activation_tables so every function present in the
    preferred set resolves only to it (same names/order, so the emitted
    act_func_set_id still indexes the real act_info.json)."""
    import functools

    @functools.cache
    def wrapper(arch):
        tables = dict(orig_fn(arch))
        pref = tables.get(preferred)
        if not pref:
            return tables
        return {
            name: (funcs if name == preferred else funcs - pref)
            for name, funcs in tables.items()
        }
    return wrapper


def _build_program(mu: float, beta: float, inv2sig2: float, norm: float, L: int):
    import concourse.bass as bass
    import concourse.bacc as bacc
    import concourse.tile as tile
    import concourse.mybir as mybir

    f32 = mybir.dt.float32
    f32r = mybir.dt.float32r
    bf16 = mybir.dt.bfloat16
    Act = mybir.ActivationFunctionType
    Op = mybir.AluOpType

    nc = bacc.Bacc("TRN2", target_bir_lowering=False, debug=False,
                   enable_asserts=True, num_devices=NCORES)

    # ---- DRAM I/O
    iblob_d = nc.dram_tensor("iblob", [128, 600], f32, kind="ExternalInput").ap()
    grid_rhs_d = nc.dram_tensor("grid_rhs", [5, HALF], f32, kind="ExternalInput").ap()
    cblob_d = nc.dram_tensor("cblob", [128, 256], f32, kind="ExternalInput").ap()
    scoef_d = nc.dram_tensor("scoef", [2, 2], f32, kind="ExternalInput").ap()
    lams_o = nc.dram_tensor("lams_o", [L], f32, kind="ExternalOutput").ap()
    ll_o = nc.dram_tensor("ll_o", [1], f32, kind="ExternalOutput").ap()

    with tile.TileContext(nc) as tc, ExitStack() as ctx:
        pool = ctx.enter_context(tc.tile_pool(name="sbuf", bufs=1))
        cpool = ctx.enter_context(tc.tile_pool(name="chunk", bufs=2))
        psum = ctx.enter_context(tc.tile_pool(name="psum", bufs=1,
                                              space=bass.MemorySpace.PSUM))
        psmall = ctx.enter_context(tc.tile_pool(name="psmall", bufs=2,
                                                space=bass.MemorySpace.PSUM))

        # ---- loads (4 DMAs on 3 queues)
        iblob = pool.tile([128, 600], f32)
        nc.sync.dma_start(iblob[:], iblob_d[:])
        cblob = pool.tile([128, 256], f32)
        nc.gpsimd.dma_start(cblob[:], cblob_d[:])
        rhs5 = pool.tile([5, HALF], f32)
        nc.scalar.dma_start(rhs5[:], grid_rhs_d[:])
        scoef = pool.tile([2, 2], f32)
        nc.gpsimd.dma_start(scoef[:], scoef_d[:])

        t_col = iblob[:, 0:1]
        s1_col = iblob[:, 1:2]
        s2_col = iblob[:, 2:3]
        t_row = iblob[0:1, 3:3 + L]
        s1_row = iblob[0:1, 3 + L:3 + 2 * L]
        s2_row = iblob[0:1, 3 + 2 * L:3 + 3 * L]
        lhsT5 = iblob[0:5, 195:323]

        tril = cblob[0:L, 0:L]
        ones_col = cblob[:, 64:65]
        sel_col = cblob[:, 65:66]
        ones_r = cblob[0:1, 66:194]
        ones_r64 = cblob[0:1, 66:130]
        negttg = cblob[0:1, 194:244]

        mu_col = pool.tile([128, 1], f32)
        nc.vector.memset(mu_col[:], mu)

        # ---- per-partition spatial bias: -inv2sig2 * (s1^2 + s2^2)
        c2sq = pool.tile([128, 1], f32)
        nc.vector.tensor_scalar(c2sq[:], s2_col, s2_col, None, Op.mult)
        ssq = pool.tile([128, 1], f32)
        nc.vector.scalar_tensor_tensor(ssq[:], s1_col, s1_col, c2sq[:],
                                       Op.mult, Op.add)
        gbias = pool.tile([128, 1], f32)
        nc.vector.tensor_scalar(gbias[:], ssq[:], -inv2sig2, None, Op.mult)

        # ---- temporal weights W_T[j(packed), r] (norm folded into mask)
        bc_ps = psmall.tile([128, R], f32, tag="small")
        nc.tensor.matmul(bc_ps[:], ones_r[:], negttg[:], start=True, stop=True)
        dtW = pool.tile([128, R], f32)
        nc.vector.tensor_scalar(dtW[:], bc_ps[:], t_col, None, Op.add)
        Ew = pool.tile([128, R], f32)
        nc.scalar.activation(Ew[:], dtW[:], Act.Exp, scale=beta)
        hn_col = pool.tile([128, 1], f32)
        nc.vector.tensor_scalar(hn_col[:], t_col, 0.0, norm, Op.is_gt, Op.mult)
        h_col = pool.tile([128, 1], f32)
        nc.vector.tensor_scalar(h_col[:], t_col, 0.0, None, Op.is_gt)
        Mw = pool.tile([128, R], f32)
        nc.vector.tensor_scalar(Mw[:], dtW[:], 0.0, hn_col[:, 0:1], Op.is_le, Op.mult)
        WT2 = pool.tile([128, 64], bf16)
        nc.vector.memset(WT2[:], 0.0)
        nc.vector.tensor_tensor(WT2[:, 0:R], Ew[:], Mw[:], Op.mult)

        # ---- per-event exponent via 2 accumulated K=2 matmuls (fp32)
        pA_in = iblob[0:2, 336:336 + L]
        pB_in = iblob[0:2, 400:400 + L]
        rhs_mm1 = iblob[0:2, 464:464 + L]
        lhsT_mm2 = iblob[0:2, 528:528 + L]

        sq1 = pool.tile([1, L], f32)
        nc.vector.tensor_tensor(sq1[:], s1_row, s1_row, Op.mult)
        sq2 = pool.tile([1, L], f32)
        nc.vector.tensor_tensor(sq2[:], s2_row, s2_row, Op.mult)
        ssum = pool.tile([1, L], f32)
        nc.vector.tensor_tensor(ssum[:], sq1[:], sq2[:], Op.add)
        w2 = pool.tile([2, L], f32)
        nc.vector.memset(w2[:], 0.0)
        nc.vector.tensor_scalar(w2[0:1, :], ssum[:], -inv2sig2, None, Op.mult)

        pairA = pool.tile([2, L], f32)
        nc.vector.scalar_tensor_tensor(pairA[:], pA_in, scoef[0:2, 0:1],
                                       w2[:], Op.mult, Op.add)
        pairB = pool.tile([2, L], f32)
        nc.vector.scalar_tensor_tensor(pairB[:], pB_in, scoef[0:2, 1:2],
                                       w2[:], Op.mult, Op.add)
        ha = pool.tile([1, L], f32)
        nc.vector.tensor_scalar(ha[:], t_row, 0.0, 1.0, Op.is_gt, Op.subtract)
        hm = pool.tile([1, L], f32)
        nc.vector.tensor_scalar(hm[:], ha[:], BIG_NEG, None, Op.mult)
        nc.vector.tensor_tensor(pairB[0:1, :], pairB[0:1, :], hm[:], Op.add)

        zev_ps = psmall.tile([L, L], f32, tag="small")
        nc.tensor.matmul(zev_ps[:], pairA[:], rhs_mm1, start=True, stop=False)
        nc.tensor.matmul(zev_ps[:], lhsT_mm2, pairB[:], start=False, stop=True)

        Ke = pool.tile([L, L], f32)
        nc.scalar.activation(Ke[:], zev_ps[:], Act.Exp)
        Km = pool.tile([L, L], f32)
        lam_col = pool.tile([L, 1], f32)
        nc.vector.scalar_tensor_tensor(Km[:], Ke[:], 0.0, tril,
                                       Op.add, Op.mult, accum_out=lam_col[:])

        # lams = softplus(lam_raw + mu) + 1e-5 ; log-lik pieces
        rr = pool.tile([L, 1], f32)
        nc.vector.tensor_scalar(rr[:], lam_col[:], mu, 0.0, Op.add, Op.max)
        tt_ = pool.tile([L, 1], f32)
        nc.vector.scalar_tensor_tensor(tt_[:], rr[:], 2.0, lam_col[:],
                                       Op.mult, Op.subtract)
        ee = pool.tile([L, 1], f32)
        nc.scalar.activation(ee[:], tt_[:], Act.Exp, scale=-1.0,
                             bias=mu_col[0:L, 0:1])
        lp = pool.tile([L, 1], f32)
        nc.scalar.activation(lp[:], ee[:], Act.Ln, bias=1.0)
        lams_col = pool.tile([L, 1], f32)
        nc.vector.scalar_tensor_tensor(lams_col[:], lp[:], 1e-5, rr[:],
                                       Op.add, Op.add)
        nc.sync.dma_start(lams_o.unsqueeze(1), lams_col[:])

        lnl = pool.tile([L, 1], f32)
        nc.scalar.activation(lnl[:], lams_col[:], Act.Ln)
        lnm = pool.tile([L, 1], f32)
        nc.vector.tensor_tensor(lnm[:], lnl[:], h_col[0:L, 0:1], Op.mult)
        red_ps = psmall.tile([1, 2], f32, tag="small")
        nc.tensor.matmul(red_ps[0:1, 0:1], lnm[:, 0:1], ones_col[0:L, 0:1],
                         start=True, stop=True)

        # ---- grid: per 512-chunk pipeline of
        #      ds2 matmul -> G=exp -> W.T@G -> softplus+accumulate
        acc = pool.tile([128, 4], f32)
        G = pool.tile([128, HALF], bf16)
        ds2_ps = psum.tile([128, 1536], f32, tag="ds2")
        z_ps = psum.tile([128, 1536], f32, tag="z")
        for ci, (off, w) in enumerate(CHUNKS):
            nc.tensor.matmul(ds2_ps[:, off:off + w], lhsT5,
                             rhs5[0:5, off:off + w], start=True, stop=True)
            nc.scalar.activation(G[:, off:off + w], ds2_ps[:, off:off + w],
                                 Act.Exp, scale=-inv2sig2, bias=gbias[:, 0:1])
            for h in (0, 1):
                p0 = h * 64
                nc.tensor.matmul(z_ps[p0:p0 + 64, off:off + w],
                                 WT2[p0:p0 + 64, 0:64],
                                 G[p0:p0 + 64, off:off + w],
                                 start=True, stop=True)
            # softplus(v)+sum, v = z+mu: r=relu(v); exp(-(2r-z-mu)); log1p; +r
            r_c = cpool.tile([128, 512], f32, tag="r_c")
            nc.vector.tensor_scalar(r_c[:, 0:w], z_ps[:, off:off + w],
                                    mu, 0.0, Op.add, Op.max)
            t_c = cpool.tile([128, 512], f32, tag="t_c")
            nc.vector.scalar_tensor_tensor(t_c[:, 0:w], r_c[:, 0:w], 2.0,
                                           z_ps[:, off:off + w],
                                           Op.mult, Op.subtract)
            e_c = cpool.tile([128, 512], f32, tag="e_c")
            nc.scalar.activation(e_c[:, 0:w], t_c[:, 0:w], Act.Exp,
                                 scale=-1.0, bias=mu_col[:, 0:1])
            l_c = cpool.tile([128, 512], f32, tag="l_c")
            nc.scalar.activation(l_c[:, 0:w], e_c[:, 0:w], Act.Ln, bias=1.0)
            sp_c = cpool.tile([128, 512], f32, tag="sp_c")
            nc.vector.scalar_tensor_tensor(sp_c[:, 0:w], l_c[:, 0:w], 0.0,
                                           r_c[:, 0:w], Op.add, Op.add,
                                           accum_out=acc[:, ci:ci + 1])

        int_col = pool.tile([128, 1], f32)
        nc.vector.tensor_reduce(int_col[:], acc[:, 0:3],
                                mybir.AxisListType.X, Op.add)
        nc.tensor.matmul(red_ps[0:1, 1:2], int_col[:, 0:1], sel_col,
                         start=True, stop=True)

        sl_sb = pool.tile([1, 1], f32)
        nc.scalar.copy(sl_sb[:], red_ps[0:1, 0:1])
        ll_sb = pool.tile([1, 1], f32)
        nc.vector.scalar_tensor_tensor(ll_sb[:], red_ps[0:1, 1:2], -UNIT_VOL,
                                       sl_sb[:], Op.mult, Op.add)
        nc.sync.dma_start(ll_o.unsqueeze(1), ll_sb[:])

    import concourse.hw_specs as hw_specs
    orig = bacc.get_activation_tables
    bacc.get_activation_tables = _patched_act_tables(hw_specs.get_activation_tables)
    try:
        nc.compile()
    finally:
        bacc.get_activation_tables = orig
    return nc


def _get_program(mu, alpha, beta, sigma, L):
    key = (float(mu), float(alpha), float(beta), float(sigma), L)
    if key not in _prog_cache:
        sig2 = float(sigma) * float(sigma)
        inv2sig2 = 1.0 / (2.0 * sig2)
        norm = float(alpha) * float(beta) / (2.0 * math.pi * sig2)
        nc = _build_program(float(mu), float(beta), inv2sig2, norm, L)
        consts = _const_arrays(L, norm)
        consts["scoef"] = _scalar_consts(float(beta), inv2sig2)
        _prog_cache[key] = (nc, consts)
    return _prog_cache[key]


def kernel(x, mu, alpha, beta, sigma):
    from concourse.bass_utils import run_bass_kernel_spmd

    x = np.asarray(x, dtype=np.float32)
    B, L, _ = x.shape
    assert B == NCORES, f"expected batch {NCORES}, got {B}"

    nc, consts = _get_program(mu, alpha, beta, sigma, L)

    in_maps = []
    for b in range(B):
        m = dict(consts)
        m.update(_marshal_core_inputs(x[b, :, 0], x[b, :, 1], x[b, :, 2]))
        in_maps.append(m)

    res = run_bass_kernel_spmd(nc, in_maps, list(range(NCORES)))
    lams = np.stack([res.results[b]["lams_o"] for b in range(B)]).astype(np.float32)
    loglik = np.stack([res.results[b]["ll_o"][0] for b in range(B)]).astype(np.float32)
    return lams, loglik
